# revision 1
# baseline (speedup 1.0000x reference)
"""Trainium2 Bass kernel: BinConv(3x3, pad 1) + BatchNorm(train) + Hardtanh.

Data-parallel over the batch across 8 NeuronCores (4 images/core), weights and
BN params replicated; BN batch statistics cross-core via AllGather + local
reduce. See build() docstring notes inline.

Key design points:
  - binarize x and W to +-0.5 in one DVE op (is_ge; subtract 0.5). BatchNorm is
    positively scale-invariant, so conv(+-0.5, +-0.5) = conv(+-1, +-1)/4
    normalizes identically (eps enters at var/16: ~1e-7 relative effect).
  - +-0.5 is exact in fp8e4; matmuls run fp8 with MatmulPerfMode.DoubleRow so
    one matmul contracts all 256 input channels (2 fp8 weights per PE cell).
  - activations live in SBUF as flat zero-padded 58x58 frames [c=128, 2, 3376]
    (3376 = pad for DoubleRow's 16-byte half-stride rule). A PSUM tile of
    [128, 464] covers 8 output rows in padded-frame coords (8*58), which makes
    every conv tap's rhs window contiguous (offset (8ch+dy)*58+dx); the two
    garbage columns per row are never evicted.
  - conv outputs are quarter-integers <= 576 -> exactly representable in fp16;
    y is staged in SBUF fp16 between pass 1 and pass 2.
  - per-chunk sum/sumsq stats are fused into PSUM eviction via accum_out
    (DVE copy for sum, ACT Square for sumsq).
  - host passes x/W pre-cast to bf16 (sign-exact vs f32) and W pre-laid-out as
    [c_lo=128, c_hi=2, tap=9, k]; output leaves the device as fp16 (values are
    clipped to [-1,1]; ~5e-4 quantization) and is cast to f32 on host.
  - two tiny warm-up AllGathers keep the collectives firmware awake so the
    stats collective's wake-up latency stays low.
"""


from contextlib import ExitStack

import numpy as np

import concourse.bacc as bacc
import concourse.tile as tile
from concourse import mybir

F32 = mybir.dt.float32
BF16 = mybir.dt.bfloat16
F16 = mybir.dt.float16
FP8 = mybir.dt.float8e4
AF = mybir.ActivationFunctionType
ALU = mybir.AluOpType

EPS = 1e-5
C = 256
K = 256
H = 56
HP = 58
SP = HP * HP  # 3364
SPPAD = 3376  # % 16 == 0 for DoubleRow half-stride
NCHUNK = 7  # chunks of 8 rows
ROWS = 8
WIN = ROWS * HP  # 464 contiguous window per chunk
HH = H // 2  # row-half for split loads


def build(n_cores: int, nimg: int, total_imgs: int, use_allgather: bool = True):
    """Build the per-core SPMD kernel. nimg = images per core."""
    nc = bacc.Bacc("TRN2", target_bir_lowering=False, debug=False, num_devices=n_cores)

    x_h = nc.dram_tensor("x", [nimg, C, H, H], BF16, kind="ExternalInput")
    # W host-transposed/interleaved to [c_lo=128, c_hi=2, tap=9, k=K]
    w_h = nc.dram_tensor("w", [128, 2, 9, K], BF16, kind="ExternalInput")
    gamma_h = nc.dram_tensor("gamma", [K, 1], F32, kind="ExternalInput")
    beta_h = nc.dram_tensor("beta", [K, 1], F32, kind="ExternalInput")
    out_h = nc.dram_tensor("out", [nimg, K, H, H], F16, kind="ExternalOutput")

    inv_cnt = 1.0 / float(total_imgs * H * H)

    with ExitStack() as ctx:
        tc = ctx.enter_context(tile.TileContext(nc))
        singles = ctx.enter_context(tc.tile_pool(name="singles", bufs=1))
        wtmp = ctx.enter_context(tc.tile_pool(name="wtmp", bufs=1))
        xpool = ctx.enter_context(tc.tile_pool(name="xpool", bufs=3))
        xppool = ctx.enter_context(tc.tile_pool(name="xppool", bufs=3))
        ypool = ctx.enter_context(tc.tile_pool(name="ypool", bufs=1))
        evpool = ctx.enter_context(tc.tile_pool(name="evpool", bufs=3))
        obpool = ctx.enter_context(tc.tile_pool(name="obpool", bufs=4))
        psum = ctx.enter_context(tc.tile_pool(name="psum", bufs=8, space="PSUM"))
        dram = ctx.enter_context(tc.tile_pool(name="dram", bufs=1, space="DRAM"))

        # W gates the first matmul: issue on sync first; x q0 on scalar in parallel
        HQ = H // 4
        def load_x(n, quarters=(0, 1, 2, 3), eng=None):
            eng = eng or nc.sync
            xrs = []
            for ct in range(2):
                xr = xpool.tile([128, H, H], BF16, tag="xr", name=f"xr{n}_{ct}")
                for q in quarters:
                    eng.dma_start(
                        out=xr[:, q * HQ : (q + 1) * HQ, :],
                        in_=x_h[
                            n, ct * 128 : (ct + 1) * 128, q * HQ : (q + 1) * HQ, :
                        ],
                    )
                xrs.append(xr)
            return xrs

        wraw = wtmp.tile([128, 2, 9, K], BF16)
        wfp8 = singles.tile([128, 2, 9, K], FP8)
        wq = [(0, 0, 5), (1, 0, 5), (0, 5, 9), (1, 5, 9)]  # th0 both ci first
        for i, (ci, t0, t1) in enumerate(wq):
            eng = nc.sync if i % 2 == 0 else nc.scalar
            eng.dma_start(out=wraw[:, ci, t0:t1], in_=w_h[:, ci, t0:t1])

        xr0 = load_x(0, quarters=(0,), eng=nc.scalar)
        for ct in range(2):
            for q in (1, 2, 3):
                nc.scalar.dma_start(
                    out=xr0[ct][:, q * HQ : (q + 1) * HQ, :],
                    in_=x_h[0, ct * 128 : (ct + 1) * 128, q * HQ : (q + 1) * HQ, :],
                )

        def w_binarize(t0, t1):
            for ci in range(2):
                nc.vector.tensor_scalar(
                    out=wfp8[:, ci, t0:t1],
                    in0=wraw[:, ci, t0:t1],
                    scalar1=0.0,
                    scalar2=0.5,
                    op0=ALU.is_ge,
                    op1=ALU.subtract,
                )

        eps_t = singles.tile([128, 1], F32)
        nc.vector.memset(eps_t[:], EPS)

        # warm up ncfw so the real collective's wake-up latency is short
        warm_in = dram.tile([128, 1], F32)
        warm_out = dram.tile([n_cores * 128, 1], F32, addr_space="Shared")
        nc.sync.dma_start(out=warm_in[:], in_=gamma_h[0:128, :])
        nc.gpsimd.collective_compute(
            "AllGather",
            ALU.bypass,
            replica_groups=[list(range(n_cores))],
            ins=[warm_in.opt()],
            outs=[warm_out.opt()],
        )

        gam = singles.tile([128, 2], F32)
        bet = singles.tile([128, 2], F32)
        for kt in range(2):
            nc.sync.dma_start(
                out=gam[:, kt : kt + 1], in_=gamma_h[kt * 128 : (kt + 1) * 128, :]
            )
            nc.sync.dma_start(
                out=bet[:, kt : kt + 1], in_=beta_h[kt * 128 : (kt + 1) * 128, :]
            )

        # ---------------- pass 1: conv + stats ----------------
        ysb = [
            ypool.tile([128, nimg, NCHUNK, ROWS, H], F16, name=f"ysb{kt}")
            for kt in range(2)
        ]
        sumc = singles.tile([128, 2, nimg * NCHUNK], F32)
        sqc = singles.tile([128, 2, nimg * NCHUNK], F32)
        locp = singles.tile([128, 4, nimg], F32)

        def binarize(n, xrs):
            xp = xppool.tile([128, 2, SPPAD], FP8, tag="xp", name=f"xp{n}")
            for ct in range(2):
                v = xp[:, ct, :SP].rearrange("p (h w) -> p h w", w=HP)
                # zero border + tail pad; interior fully overwritten
                nc.vector.memset(v[:, 0, :], 0.0)
                nc.vector.memset(v[:, HP - 1, :], 0.0)
                nc.vector.memset(v[:, 1 : HP - 1, 0:1], 0.0)
                nc.vector.memset(v[:, 1 : HP - 1, HP - 1 : HP], 0.0)
                for q in range(4):
                    nc.vector.tensor_scalar(
                        out=v[:, 1 + q * HQ : 1 + (q + 1) * HQ, 1 : HP - 1],
                        in0=xrs[ct][:, q * HQ : (q + 1) * HQ, :],
                        scalar1=0.0,
                        scalar2=0.5,
                        op0=ALU.is_ge,
                        op1=ALU.subtract,
                    )
            return xp

        # image-0 prologue interleaved with W binarize so chunk-0 deps clear first
        xp0 = xppool.tile([128, 2, SPPAD], FP8, tag="xp", name="xp0")
        v0 = [xp0[:, ct, :SP].rearrange("p (h w) -> p h w", w=HP) for ct in range(2)]
        for ct in range(2):
            v = v0[ct]
            nc.vector.memset(v[:, 0, :], 0.0)
            nc.vector.memset(v[:, HP - 1, :], 0.0)
            nc.vector.memset(v[:, 1 : HP - 1, 0:1], 0.0)
            nc.vector.memset(v[:, 1 : HP - 1, HP - 1 : HP], 0.0)

        def bin_quarter(q):
            for ct in range(2):
                nc.vector.tensor_scalar(
                    out=v0[ct][:, 1 + q * HQ : 1 + (q + 1) * HQ, 1 : HP - 1],
                    in0=xr0[ct][:, q * HQ : (q + 1) * HQ, :],
                    scalar1=0.0,
                    scalar2=0.5,
                    op0=ALU.is_ge,
                    op1=ALU.subtract,
                )

        bin_quarter(0)
        w_binarize(0, 5)
        bin_quarter(1)
        bin_quarter(2)
        bin_quarter(3)
        w_binarize(5, 9)

        xp_cur = xp0
        for n in range(nimg):
            xp_use, xp_cur = xp_cur, None
            if n + 1 < nimg:
                xp_cur = binarize(n + 1, load_x(n + 1))
            if n == nimg - 2:
                # keep ncfw warm so the real collective wakes fast
                warm2_in = dram.tile([128, 1], F32)
                warm2_out = dram.tile(
                    [n_cores * 128, 1], F32, addr_space="Shared", name="warm2_out"
                )
                nc.sync.dma_start(out=warm2_in[:], in_=gamma_h[0:128, :])
                nc.gpsimd.collective_compute(
                    "AllGather",
                    ALU.bypass,
                    replica_groups=[list(range(n_cores))],
                    ins=[warm2_in.opt()],
                    outs=[warm2_out.opt()],
                )

            xpv = xp_use[:, :, :SP].rearrange("p i (h w) -> p i h w", w=HP)
            for kt in range(2):
                banks = [
                    psum.tile([128, ROWS, H], F32, tag="ps", name=f"ps{n}_{kt}_{ch}")
                    for ch in range(NCHUNK)
                ]
                for dy in range(3):
                    for dx in range(3):
                        t9 = dy * 3 + dx
                        for ch in range(NCHUNK):
                            r0 = ROWS * ch + dy
                            nc.tensor.matmul(
                                banks[ch][:],
                                wfp8[:, :, t9, kt * 128 : (kt + 1) * 128],
                                xpv[:, :, r0 : r0 + ROWS, dx : dx + H],
                                start=(t9 == 0),
                                stop=(t9 == 8),
                                perf_mode=mybir.MatmulPerfMode.DoubleRow,
                            )
                for ch in range(NCHUNK):
                    col = n * NCHUNK + ch
                    psv = banks[ch][:]
                    # evict valid columns to fp16 (exact) + per-chunk sum on DVE
                    nc.vector.tensor_scalar(
                        out=ysb[kt][:, n, ch],
                        in0=psv,
                        scalar1=1.0,
                        scalar2=0.0,
                        op0=ALU.mult,
                        op1=ALU.add,
                        accum_out=sumc[:, kt, col : col + 1],
                    )
                    # sum of squares on ACT
                    sqs = evpool.tile([128, ROWS, H], F32, tag="sqs")
                    nc.scalar.activation(
                        out=sqs[:],
                        in_=psv,
                        func=AF.Square,
                        accum_out=sqc[:, kt, col : col + 1],
                    )

            # fold this image's 7-chunk partials; hidden under the next image
            pscr = evpool.tile([128, 2, NCHUNK], F32, tag="pscr")
            for kt in range(2):
                nc.vector.tensor_scalar(
                    out=pscr[:, kt],
                    in0=sumc[:, kt, n * NCHUNK : (n + 1) * NCHUNK],
                    scalar1=1.0,
                    scalar2=0.0,
                    op0=ALU.mult,
                    op1=ALU.add,
                    accum_out=locp[:, kt, n : n + 1],
                )
                nc.vector.tensor_scalar(
                    out=pscr[:, kt],
                    in0=sqc[:, kt, n * NCHUNK : (n + 1) * NCHUNK],
                    scalar1=1.0,
                    scalar2=0.0,
                    op0=ALU.mult,
                    op1=ALU.add,
                    accum_out=locp[:, 2 + kt, n : n + 1],
                )

        # ---------------- stats reduce + collective ----------------
        loc = singles.tile([128, 4], F32)
        fscr = singles.tile([128, 4, nimg], F32)
        for s in range(4):
            nc.vector.tensor_scalar(
                out=fscr[:, s],
                in0=locp[:, s],
                scalar1=1.0,
                scalar2=0.0,
                op0=ALU.mult,
                op1=ALU.add,
                accum_out=loc[:, s : s + 1],
            )

        gstat = singles.tile([128, 4], F32)
        if use_allgather:
            cc_in = dram.tile([128, 4], F32)
            cc_out = dram.tile([n_cores * 128, 4], F32, addr_space="Shared")
            nc.sync.dma_start(out=cc_in[:], in_=loc[:])
            nc.gpsimd.collective_compute(
                "AllGather",
                ALU.bypass,
                replica_groups=[list(range(n_cores))],
                ins=[cc_in.opt()],
                outs=[cc_out.opt()],
            )
            # gather all ranks' stats to SBUF then reduce locally
            allst = singles.tile([128, n_cores, 4], F32)
            nc.sync.dma_start(
                out=allst[:],
                in_=cc_out.rearrange("(r p) c -> p r c", p=128),
            )
            h = n_cores // 2
            while h > 1:
                nc.vector.tensor_add(
                    out=allst[:, 0:h, :], in0=allst[:, 0:h, :], in1=allst[:, h : 2 * h, :]
                )
                h //= 2
            nc.vector.tensor_add(
                out=gstat[:], in0=allst[:, 0, :], in1=allst[:, 1, :]
            )
        else:
            cc_in = dram.tile([128, 4], F32)
            cc_out = dram.tile([128, 4], F32, addr_space="Shared")
            nc.sync.dma_start(out=cc_in[:], in_=loc[:])
            nc.gpsimd.collective_compute(
                "AllReduce",
                ALU.add,
                replica_groups=[list(range(n_cores))],
                ins=[cc_in.opt()],
                outs=[cc_out.opt()],
            )
            nc.sync.dma_start(out=gstat[:], in_=cc_out[:])

        # ---------------- scale/bias ----------------
        mv = singles.tile([128, 4], F32)
        nc.vector.tensor_scalar(
            out=mv[:], in0=gstat[:], scalar1=inv_cnt, scalar2=None, op0=ALU.mult
        )
        mean = mv[:, 0:2]
        var = mv[:, 2:4]
        m2 = singles.tile([128, 2], F32)
        nc.vector.tensor_mul(out=m2[:], in0=mean, in1=mean)
        nc.vector.tensor_sub(out=var, in0=var, in1=m2[:])
        rstd = singles.tile([128, 2], F32)
        nc.scalar.activation(
            out=rstd[:], in_=var, func=AF.Abs_reciprocal_sqrt, bias=eps_t[:]
        )
        scl = singles.tile([128, 2], F32)
        nc.vector.tensor_mul(out=scl[:], in0=gam[:], in1=rstd[:])
        bia = singles.tile([128, 2], F32)
        nc.vector.tensor_mul(out=bia[:], in0=mean[:], in1=scl[:])
        nc.vector.tensor_sub(out=bia[:], in0=bet[:], in1=bia[:])

        # ---------------- pass 2: affine + clip + DMA out ----------------
        NHALF = NCHUNK * ROWS * H // 2
        unit = 0
        for n in range(nimg):
            for kt in range(2):
                ob = obpool.tile([128, NCHUNK * ROWS * H], F16, tag="ob")
                ysrc = ysb[kt][:, n].rearrange("p a b c -> p (a b c)")
                if unit == 0:
                    # split the first unit so output writes start sooner
                    nc.vector.tensor_scalar(
                        out=ob[:, :NHALF],
                        in0=ysrc[:, :NHALF],
                        scalar1=scl[:, kt : kt + 1],
                        scalar2=bia[:, kt : kt + 1],
                        op0=ALU.mult,
                        op1=ALU.add,
                    )
                    nc.scalar.activation(
                        out=ob[:, NHALF:],
                        in_=ysrc[:, NHALF:],
                        func=AF.Identity,
                        bias=bia[:, kt : kt + 1],
                        scale=scl[:, kt : kt + 1],
                    )
                elif unit % 8 in (1, 4, 7):
                    nc.vector.tensor_scalar(
                        out=ob[:],
                        in0=ysrc,
                        scalar1=scl[:, kt : kt + 1],
                        scalar2=bia[:, kt : kt + 1],
                        op0=ALU.mult,
                        op1=ALU.add,
                    )
                else:
                    nc.scalar.activation(
                        out=ob[:],
                        in_=ysrc,
                        func=AF.Identity,
                        bias=bia[:, kt : kt + 1],
                        scale=scl[:, kt : kt + 1],
                    )
                obv = ob[:].rearrange("p (a b) -> p a b", b=H)
                for hf in range(2):
                    sl = slice(hf * NHALF, (hf + 1) * NHALF)
                    clip_eng = nc.vector if hf == 0 else nc.gpsimd
                    clip_eng.tensor_scalar(
                        out=ob[:, sl],
                        in0=ob[:, sl],
                        scalar1=1.0,
                        scalar2=-1.0,
                        op0=ALU.min,
                        op1=ALU.max,
                    )
                    dma_eng = nc.sync if hf == 0 else nc.scalar
                    dma_eng.dma_start(
                        out=out_h[
                            n,
                            kt * 128 : (kt + 1) * 128,
                            hf * (H // 2) : (hf + 1) * (H // 2),
                            :,
                        ],
                        in_=obv[:, hf * (H // 2) : (hf + 1) * (H // 2), :],
                    )
                unit += 1

    nc.compile()
    return nc


def prep_w(W):
    """Host layout prep: W [K,C,3,3] -> [c_lo=128, c_hi=2, tap=9, K] f32."""
    wt = W.astype(np.float32).transpose(1, 2, 3, 0).reshape(C, 9, K)  # [c, t, k]
    return np.ascontiguousarray(wt.reshape(2, 128, 9, K).transpose(1, 0, 2, 3))


def _ensure_ntff_hooks():
    """Make run_bass_kernel_spmd's trace path importable on images whose
    antenv lacks axon_hooks (bass_utils hard-imports it when BASS_TRACE is
    set). Registers the real ctypes hook when available, else a None hook
    (bass_utils then logs and skips tracing instead of crashing)."""
    import sys
    import types

    try:
        import antenv
    except ImportError:
        return
    if hasattr(antenv, "axon_hooks") or "antenv.axon_hooks" in sys.modules:
        return
    hook = None
    try:
        from trn_agent_boot.trn_boot import _ntff_profile_via_ctypes

        hook = _ntff_profile_via_ctypes("/opt/axon/libaxon_pjrt.so")
    except Exception:
        hook = None
    mod = types.ModuleType("antenv.axon_hooks")
    mod.get_axon_ntff_profile_hook = lambda: hook
    mod.set_axon_ntff_profile_hook = lambda h: None
    sys.modules["antenv.axon_hooks"] = mod
    antenv.axon_hooks = mod


_ensure_ntff_hooks()


_CACHE = {}


def _get_compiled():
    if "nc" not in _CACHE:
        _CACHE["nc"] = build(8, 4, 32)
    return _CACHE["nc"]


def kernel(x, W, gamma, beta):
    """Full-input entry point: shard batch over 8 cores, run SPMD, gather."""
    import ml_dtypes
    from concourse.bass_utils import run_bass_kernel_spmd

    n_cores, nimg = 8, 4
    nc = _get_compiled()
    w2 = prep_w(np.asarray(W)).astype(ml_dtypes.bfloat16)
    g2 = np.ascontiguousarray(np.asarray(gamma, np.float32).reshape(K, 1))
    b2 = np.ascontiguousarray(np.asarray(beta, np.float32).reshape(K, 1))
    xb = np.asarray(x).astype(ml_dtypes.bfloat16)
    in_maps = [
        {
            "x": np.ascontiguousarray(xb[c * nimg : (c + 1) * nimg]),
            "w": w2,
            "gamma": g2,
            "beta": b2,
        }
        for c in range(n_cores)
    ]
    res = run_bass_kernel_spmd(nc, in_maps, core_ids=list(range(n_cores)))
    out = np.concatenate(
        [res.results[c]["out"] for c in range(n_cores)], axis=0
    ).astype(np.float32)
    return out


def run_traced(x, W, gamma, beta):
    """Like kernel() but with NTFF tracing; returns (out, BassKernelResults)."""
    import ml_dtypes
    from concourse.bass_utils import run_bass_kernel_spmd

    n_cores, nimg = 8, 4
    nc = _get_compiled()
    w2 = prep_w(np.asarray(W)).astype(ml_dtypes.bfloat16)
    g2 = np.ascontiguousarray(np.asarray(gamma, np.float32).reshape(K, 1))
    b2 = np.ascontiguousarray(np.asarray(beta, np.float32).reshape(K, 1))
    xb = np.asarray(x).astype(ml_dtypes.bfloat16)
    in_maps = [
        {
            "x": np.ascontiguousarray(xb[c * nimg : (c + 1) * nimg]),
            "w": w2,
            "gamma": g2,
            "beta": b2,
        }
        for c in range(n_cores)
    ]
    res = run_bass_kernel_spmd(nc, in_maps, core_ids=list(range(n_cores)), trace=True)
    out = np.concatenate(
        [res.results[c]["out"] for c in range(n_cores)], axis=0
    ).astype(np.float32)
    return out, res



# revision 4
# speedup vs baseline: 1.0060x; 1.0060x over previous
"""Trainium2 Bass kernel: BinConv(3x3, pad 1) + BatchNorm(train) + Hardtanh.

Data-parallel over the batch across 8 NeuronCores (4 images/core), weights and
BN params replicated; BN batch statistics cross-core via AllGather + local
reduce.

Key design points:
  - binarize x and W to +-0.5 in one DVE op (is_ge; subtract 0.5). BatchNorm is
    positively scale-invariant, so conv(+-0.5, +-0.5) = conv(+-1, +-1)/4
    normalizes identically (eps enters at var/16: ~1e-7 relative effect).
  - +-0.5 is exact in fp8e4; matmuls run fp8 with MatmulPerfMode.DoubleRow so
    one matmul contracts all 256 input channels (2 fp8 weights per PE cell).
  - activations live in SBUF as flat zero-padded 58x58 frames [c=128, 2, 3376]
    (3376 = pad for DoubleRow's 16-byte half-stride rule). A PSUM tile of
    [128, 464] covers 8 output rows in padded-frame coords (8*58), which makes
    every conv tap's rhs window contiguous (offset (8ch+dy)*58+dx); the two
    garbage columns per row are never evicted.
  - conv outputs are quarter-integers <= 576 -> exactly representable in fp16;
    y is staged in SBUF fp16 between pass 1 and pass 2.
  - per-chunk sum/sumsq stats are fused into PSUM eviction via accum_out
    (DVE copy for sum, ACT Square for sumsq).
  - W loads are split per-tap-group so the first matmul can start as soon as
    tap 0 and the first x quarter land (~7us instead of ~18us).
  - collectives: tiny warm-up AllGathers with *clean* dependency chains (own
    DRAM pools, triggered off ysb writes, all DMAs on the otherwise-idle
    gpsimd queue) keep the CC engine hot; the real stats AllGather's input
    DMA + trigger + gather also run on gpsimd so they never queue behind
    x-load triggers.
  - stats math stays on DVE except the rsqrt (ACT); the ACT table for
    Abs_reciprocal_sqrt is pre-warmed right after pass 1 so no table load
    lands on the critical path.
  - pass 2 balances affine+clip across DVE/ACT/GPSIMD by measured rates and
    issues one whole-frame output DMA per (img,kt) unit, alternating between
    the idle sync and tensor queues; the first unit is split in halves so
    output DMA starts ~1us into pass 2 (the pass is DMA-bandwidth-bound).
  - host passes x/W pre-cast to bf16 (sign-exact vs f32) and W pre-laid-out as
    [c_lo=128, c_hi=2, tap=9, k]; output leaves the device as fp16 (values are
    clipped to [-1,1]; ~5e-4 quantization) and is cast to f32 on host.
"""


from contextlib import ExitStack

import numpy as np

import concourse.bacc as bacc
import concourse.tile as tile
from concourse import mybir

F32 = mybir.dt.float32
BF16 = mybir.dt.bfloat16
F16 = mybir.dt.float16
FP8 = mybir.dt.float8e4
AF = mybir.ActivationFunctionType
ALU = mybir.AluOpType

EPS = 1e-5
C = 256
K = 256
H = 56
HP = 58
SP = HP * HP  # 3364
SPPAD = 3376  # % 16 == 0 for DoubleRow half-stride
NCHUNK = 7  # chunks of 8 rows
ROWS = 8
WIN = ROWS * HP  # 464 contiguous window per chunk
HH = H // 2  # row-half for split loads


def build(n_cores: int, nimg: int, total_imgs: int, use_allgather: bool = True):
    """Build the per-core SPMD kernel. nimg = images per core."""
    nc = bacc.Bacc("TRN2", target_bir_lowering=False, debug=False, num_devices=n_cores)

    x_h = nc.dram_tensor("x", [nimg, C, H, H], BF16, kind="ExternalInput")
    # W host-transposed/interleaved to [c_lo=128, c_hi=2, tap=9, k=K]
    w_h = nc.dram_tensor("w", [128, 2, 9, K], BF16, kind="ExternalInput")
    gamma_h = nc.dram_tensor("gamma", [K, 1], F32, kind="ExternalInput")
    beta_h = nc.dram_tensor("beta", [K, 1], F32, kind="ExternalInput")
    out_h = nc.dram_tensor("out", [nimg, K, H, H], F16, kind="ExternalOutput")

    inv_cnt = 1.0 / float(total_imgs * H * H)

    with ExitStack() as ctx:
        tc = ctx.enter_context(tile.TileContext(nc))
        singles = ctx.enter_context(tc.tile_pool(name="singles", bufs=1))
        wtmp = ctx.enter_context(tc.tile_pool(name="wtmp", bufs=1))
        xpool = ctx.enter_context(tc.tile_pool(name="xpool", bufs=3))
        xppool = ctx.enter_context(tc.tile_pool(name="xppool", bufs=3))
        ypool = ctx.enter_context(tc.tile_pool(name="ypool", bufs=1))
        evpool = ctx.enter_context(tc.tile_pool(name="evpool", bufs=3))
        obpool = ctx.enter_context(tc.tile_pool(name="obpool", bufs=4))
        psum = ctx.enter_context(tc.tile_pool(name="psum", bufs=8, space="PSUM"))
        warmd = ctx.enter_context(tc.tile_pool(name="warmd", bufs=6, space="DRAM"))
        ccd = ctx.enter_context(tc.tile_pool(name="ccd", bufs=2, space="DRAM"))

        HQ = H // 4

        # ---- startup: fine-grained W loads so tap 0 lands first ----
        wraw = wtmp.tile([128, 2, 9, K], BF16)
        wfp8 = singles.tile([128, 2, 9, K], FP8)
        WGRP = [(0, 1), (1, 5), (5, 9)]
        for t0, t1 in WGRP:
            for ci in range(2):
                nc.sync.dma_start(out=wraw[:, ci, t0:t1], in_=w_h[:, ci, t0:t1])

        # x image 0 on the scalar queue, q0 first
        xr0 = []
        for ct in range(2):
            xr = xpool.tile([128, H, H], BF16, tag="xr", name=f"xr0_{ct}")
            nc.scalar.dma_start(
                out=xr[:, 0:HQ, :], in_=x_h[0, ct * 128 : (ct + 1) * 128, 0:HQ, :]
            )
            xr0.append(xr)
        for q in (1, 2, 3):
            for ct in range(2):
                nc.scalar.dma_start(
                    out=xr0[ct][:, q * HQ : (q + 1) * HQ, :],
                    in_=x_h[0, ct * 128 : (ct + 1) * 128, q * HQ : (q + 1) * HQ, :],
                )

        def load_x(n, eng=None):
            eng = eng or nc.sync
            xrs = []
            for ct in range(2):
                xr = xpool.tile([128, H, H], BF16, tag="xr", name=f"xr{n}_{ct}")
                for q in range(4):
                    eng.dma_start(
                        out=xr[:, q * HQ : (q + 1) * HQ, :],
                        in_=x_h[
                            n, ct * 128 : (ct + 1) * 128, q * HQ : (q + 1) * HQ, :
                        ],
                    )
                xrs.append(xr)
            return xrs

        def w_binarize(t0, t1):
            for ci in range(2):
                nc.vector.tensor_scalar(
                    out=wfp8[:, ci, t0:t1],
                    in0=wraw[:, ci, t0:t1],
                    scalar1=0.0,
                    scalar2=0.5,
                    op0=ALU.is_ge,
                    op1=ALU.subtract,
                )

        # image-0 frame: borders zeroed first (pure DVE, no DMA deps)
        xp0 = xppool.tile([128, 2, SPPAD], FP8, tag="xp", name="xp0")
        v0 = [xp0[:, ct, :SP].rearrange("p (h w) -> p h w", w=HP) for ct in range(2)]
        for ct in range(2):
            v = v0[ct]
            nc.vector.memset(v[:, 0, :], 0.0)
            nc.vector.memset(v[:, HP - 1, :], 0.0)
            nc.vector.memset(v[:, 1 : HP - 1, 0:1], 0.0)
            nc.vector.memset(v[:, 1 : HP - 1, HP - 1 : HP], 0.0)

        def bin_quarter(q):
            for ct in range(2):
                nc.vector.tensor_scalar(
                    out=v0[ct][:, 1 + q * HQ : 1 + (q + 1) * HQ, 1 : HP - 1],
                    in0=xr0[ct][:, q * HQ : (q + 1) * HQ, :],
                    scalar1=0.0,
                    scalar2=0.5,
                    op0=ALU.is_ge,
                    op1=ALU.subtract,
                )

        # DVE startup order: tap-0 weights, then first x quarter, then the rest
        w_binarize(0, 1)
        bin_quarter(0)
        w_binarize(1, 5)
        bin_quarter(1)
        w_binarize(5, 9)
        bin_quarter(2)
        bin_quarter(3)

        eps_t = singles.tile([128, 1], F32)
        nc.vector.memset(eps_t[:], EPS)

        # warm-up 1: keep CC engine awake; all deps local to gpsimd/vector
        wsrc = singles.tile([128, 1], F32)
        nc.vector.memset(wsrc[:], 0.0)
        w1_in = warmd.tile([128, 1], F32)
        w1_out = warmd.tile([n_cores * 128, 1], F32, addr_space="Shared")
        nc.gpsimd.dma_start(out=w1_in[:], in_=wsrc[:])
        nc.gpsimd.collective_compute(
            "AllGather",
            ALU.bypass,
            replica_groups=[list(range(n_cores))],
            ins=[w1_in.opt()],
            outs=[w1_out.opt()],
        )

        gam = singles.tile([128, 2], F32)
        bet = singles.tile([128, 2], F32)
        for kt in range(2):
            nc.gpsimd.dma_start(
                out=gam[:, kt : kt + 1], in_=gamma_h[kt * 128 : (kt + 1) * 128, :]
            )
            nc.gpsimd.dma_start(
                out=bet[:, kt : kt + 1], in_=beta_h[kt * 128 : (kt + 1) * 128, :]
            )

        # ---------------- pass 1: conv + stats ----------------
        ysb = [
            ypool.tile([128, nimg, NCHUNK, ROWS, H], F16, name=f"ysb{kt}")
            for kt in range(2)
        ]
        sumc = singles.tile([128, 2, nimg * NCHUNK], F32)
        sqc = singles.tile([128, 2, nimg * NCHUNK], F32)

        def binarize(n, xrs):
            xp = xppool.tile([128, 2, SPPAD], FP8, tag="xp", name=f"xp{n}")
            for ct in range(2):
                v = xp[:, ct, :SP].rearrange("p (h w) -> p h w", w=HP)
                # zero border + tail pad; interior fully overwritten
                nc.vector.memset(v[:, 0, :], 0.0)
                nc.vector.memset(v[:, HP - 1, :], 0.0)
                nc.vector.memset(v[:, 1 : HP - 1, 0:1], 0.0)
                nc.vector.memset(v[:, 1 : HP - 1, HP - 1 : HP], 0.0)
                for q in range(4):
                    nc.vector.tensor_scalar(
                        out=v[:, 1 + q * HQ : 1 + (q + 1) * HQ, 1 : HP - 1],
                        in0=xrs[ct][:, q * HQ : (q + 1) * HQ, :],
                        scalar1=0.0,
                        scalar2=0.5,
                        op0=ALU.is_ge,
                        op1=ALU.subtract,
                    )
            return xp

        def issue_warm(idx, src_ap):
            """Tiny AllGather fed by a pass-1 ysb value: fires mid-pass-1,
            keeping the CC engine hot with no cross-queue entanglement."""
            win = warmd.tile([128, 1], F16, name=f"warm{idx}_in")
            wout = warmd.tile(
                [n_cores * 128, 1], F16, addr_space="Shared", name=f"warm{idx}_out"
            )
            nc.gpsimd.dma_start(out=win[:], in_=src_ap)
            nc.gpsimd.collective_compute(
                "AllGather",
                ALU.bypass,
                replica_groups=[list(range(n_cores))],
                ins=[win.opt()],
                outs=[wout.opt()],
            )

        xp_cur = xp0
        for n in range(nimg):
            xp_use, xp_cur = xp_cur, None
            if n + 1 < nimg:
                xp_cur = binarize(n + 1, load_x(n + 1))

            xpv = xp_use[:, :, :SP].rearrange("p i (h w) -> p i h w", w=HP)
            for kt in range(2):
                banks = [
                    psum.tile([128, ROWS, H], F32, tag="ps", name=f"ps{n}_{kt}_{ch}")
                    for ch in range(NCHUNK)
                ]
                for dy in range(3):
                    for dx in range(3):
                        t9 = dy * 3 + dx
                        for ch in range(NCHUNK):
                            r0 = ROWS * ch + dy
                            nc.tensor.matmul(
                                banks[ch][:],
                                wfp8[:, :, t9, kt * 128 : (kt + 1) * 128],
                                xpv[:, :, r0 : r0 + ROWS, dx : dx + H],
                                start=(t9 == 0),
                                stop=(t9 == 8),
                                perf_mode=mybir.MatmulPerfMode.DoubleRow,
                            )
                for ch in range(NCHUNK):
                    col = n * NCHUNK + ch
                    psv = banks[ch][:]
                    # evict valid columns to fp16 (exact) + per-chunk sum on DVE
                    nc.vector.tensor_scalar(
                        out=ysb[kt][:, n, ch],
                        in0=psv,
                        scalar1=1.0,
                        scalar2=0.0,
                        op0=ALU.mult,
                        op1=ALU.add,
                        accum_out=sumc[:, kt, col : col + 1],
                    )
                    # sum of squares on ACT
                    sqs = evpool.tile([128, ROWS, H], F32, tag="sqs")
                    nc.scalar.activation(
                        out=sqs[:],
                        in_=psv,
                        func=AF.Square,
                        accum_out=sqc[:, kt, col : col + 1],
                    )

            if n == 1:
                # fires once image 2's first chunk is evicted (~60% into pass 1)
                issue_warm(2, ysb[0][:, 1, 0, 0, 0:1])
            elif n == 2:
                # fires once image 3's first chunk is evicted (~85% into pass 1)
                issue_warm(3, ysb[0][:, 2, 0, 0, 0:1])

        # pre-warm the ACT rsqrt table while the collective is in flight
        tblw = singles.tile([128, 1], F32)
        nc.scalar.activation(
            out=tblw[:], in_=eps_t[:], func=AF.Abs_reciprocal_sqrt, bias=eps_t[:]
        )

        # ---------------- stats reduce + collective ----------------
        loc = singles.tile([128, 4], F32)
        fold_scr = singles.tile([128, nimg * NCHUNK], F32)
        for kt in range(2):
            nc.vector.tensor_scalar(
                out=fold_scr[:],
                in0=sumc[:, kt, :],
                scalar1=1.0,
                scalar2=0.0,
                op0=ALU.mult,
                op1=ALU.add,
                accum_out=loc[:, kt : kt + 1],
            )
            nc.vector.tensor_scalar(
                out=fold_scr[:],
                in0=sqc[:, kt, :],
                scalar1=1.0,
                scalar2=0.0,
                op0=ALU.mult,
                op1=ALU.add,
                accum_out=loc[:, 2 + kt : 3 + kt],
            )

        gstat = singles.tile([128, 4], F32)
        if use_allgather:
            cc_in = ccd.tile([128, 4], F32)
            cc_out = ccd.tile([n_cores * 128, 4], F32, addr_space="Shared")
            nc.gpsimd.dma_start(out=cc_in[:], in_=loc[:])
            nc.gpsimd.collective_compute(
                "AllGather",
                ALU.bypass,
                replica_groups=[list(range(n_cores))],
                ins=[cc_in.opt()],
                outs=[cc_out.opt()],
            )
            # gather all ranks' stats to SBUF then reduce locally
            allst = singles.tile([128, n_cores, 4], F32)
            nc.gpsimd.dma_start(
                out=allst[:],
                in_=cc_out.rearrange("(r p) c -> p r c", p=128),
            )
            h = n_cores // 2
            while h > 1:
                nc.vector.tensor_add(
                    out=allst[:, 0:h, :], in0=allst[:, 0:h, :], in1=allst[:, h : 2 * h, :]
                )
                h //= 2
            nc.vector.tensor_add(
                out=gstat[:], in0=allst[:, 0, :], in1=allst[:, 1, :]
            )
        else:
            cc_in = ccd.tile([128, 4], F32)
            cc_out = ccd.tile([128, 4], F32, addr_space="Shared")
            nc.gpsimd.dma_start(out=cc_in[:], in_=loc[:])
            nc.gpsimd.collective_compute(
                "AllReduce",
                ALU.add,
                replica_groups=[list(range(n_cores))],
                ins=[cc_in.opt()],
                outs=[cc_out.opt()],
            )
            nc.gpsimd.dma_start(out=gstat[:], in_=cc_out[:])

        # ---------------- scale/bias (DVE except the rsqrt) ----------------
        mv = singles.tile([128, 4], F32)
        nc.vector.tensor_scalar(
            out=mv[:], in0=gstat[:], scalar1=inv_cnt, scalar2=None, op0=ALU.mult
        )
        mean = mv[:, 0:2]
        var = mv[:, 2:4]
        m2 = singles.tile([128, 2], F32)
        nc.vector.tensor_mul(out=m2[:], in0=mean, in1=mean)
        nc.vector.tensor_sub(out=var, in0=var, in1=m2[:])
        rstd = singles.tile([128, 2], F32)
        nc.scalar.activation(
            out=rstd[:], in_=var, func=AF.Abs_reciprocal_sqrt, bias=eps_t[:]
        )
        scl = singles.tile([128, 2], F32)
        nc.vector.tensor_mul(out=scl[:], in0=gam[:], in1=rstd[:])
        bia = singles.tile([128, 2], F32)
        nc.vector.tensor_mul(out=bia[:], in0=mean[:], in1=scl[:])
        nc.vector.tensor_sub(out=bia[:], in0=bet[:], in1=bia[:])

        # ---------------- pass 2: affine + clip + DMA out ----------------
        # 8 units of [128, 3136] fp16; 2 elementwise passes each (affine, clip).
        # Rates (ns/elem/lane): DVE 0.36, ACT 0.91, GPS 1.16 -> DVE 9 passes,
        # ACT 4 affines, GPS 3 clips. DMA (6.4MB out, ~18us) is the pacer, so
        # unit 0 is halved to start it early; whole-frame DMAs all go on the
        # sync queue (idle in pass 2; triggers are ~0.6us vs ~2.2us transfers).
        NFULL = NCHUNK * ROWS * H
        NHALF = NFULL // 2
        aff_dve = {0, 2, 4, 6}
        clip_gps = {1, 3, 5}
        unit = 0
        for n in range(nimg):
            for kt in range(2):
                ob = obpool.tile([128, NFULL], F16, tag="ob")
                ysrc = ysb[kt][:, n].rearrange("p a b c -> p (a b c)")
                obv = ob[:].rearrange("p (a b) -> p a b", b=H)
                dma_eng = nc.sync
                if unit == 0:
                    # halves so the first output DMA fires ~1us into pass 2
                    for hf in range(2):
                        sl = slice(hf * NHALF, (hf + 1) * NHALF)
                        nc.vector.tensor_scalar(
                            out=ob[:, sl],
                            in0=ysrc[:, sl],
                            scalar1=scl[:, kt : kt + 1],
                            scalar2=bia[:, kt : kt + 1],
                            op0=ALU.mult,
                            op1=ALU.add,
                        )
                        nc.vector.tensor_scalar(
                            out=ob[:, sl],
                            in0=ob[:, sl],
                            scalar1=1.0,
                            scalar2=-1.0,
                            op0=ALU.min,
                            op1=ALU.max,
                        )
                        dma_eng.dma_start(
                            out=out_h[
                                n,
                                kt * 128 : (kt + 1) * 128,
                                hf * HH : (hf + 1) * HH,
                                :,
                            ],
                            in_=obv[:, hf * HH : (hf + 1) * HH, :],
                        )
                    unit += 1
                    continue
                if unit in aff_dve:
                    nc.vector.tensor_scalar(
                        out=ob[:],
                        in0=ysrc,
                        scalar1=scl[:, kt : kt + 1],
                        scalar2=bia[:, kt : kt + 1],
                        op0=ALU.mult,
                        op1=ALU.add,
                    )
                else:
                    nc.scalar.activation(
                        out=ob[:],
                        in_=ysrc,
                        func=AF.Identity,
                        bias=bia[:, kt : kt + 1],
                        scale=scl[:, kt : kt + 1],
                    )
                clip_eng = nc.gpsimd if unit in clip_gps else nc.vector
                clip_eng.tensor_scalar(
                    out=ob[:],
                    in0=ob[:],
                    scalar1=1.0,
                    scalar2=-1.0,
                    op0=ALU.min,
                    op1=ALU.max,
                )
                dma_eng.dma_start(
                    out=out_h[n, kt * 128 : (kt + 1) * 128, :, :],
                    in_=obv[:],
                )
                unit += 1

    nc.compile()
    return nc


def prep_w(W):
    """Host layout prep: W [K,C,3,3] -> [c_lo=128, c_hi=2, tap=9, K] f32."""
    wt = W.astype(np.float32).transpose(1, 2, 3, 0).reshape(C, 9, K)  # [c, t, k]
    return np.ascontiguousarray(wt.reshape(2, 128, 9, K).transpose(1, 0, 2, 3))


def _ensure_ntff_hooks():
    """Make run_bass_kernel_spmd's trace path importable on images whose
    antenv lacks axon_hooks (bass_utils hard-imports it when BASS_TRACE is
    set). Registers the real ctypes hook when available, else a None hook
    (bass_utils then logs and skips tracing instead of crashing)."""
    import sys
    import types

    try:
        import antenv
    except ImportError:
        return
    if hasattr(antenv, "axon_hooks") or "antenv.axon_hooks" in sys.modules:
        return
    hook = None
    try:
        from trn_agent_boot.trn_boot import _ntff_profile_via_ctypes

        hook = _ntff_profile_via_ctypes("/opt/axon/libaxon_pjrt.so")
    except Exception:
        hook = None
    mod = types.ModuleType("antenv.axon_hooks")
    mod.get_axon_ntff_profile_hook = lambda: hook
    mod.set_axon_ntff_profile_hook = lambda h: None
    sys.modules["antenv.axon_hooks"] = mod
    antenv.axon_hooks = mod


_ensure_ntff_hooks()


_CACHE = {}


def _get_compiled():
    if "nc" not in _CACHE:
        _CACHE["nc"] = build(8, 4, 32)
    return _CACHE["nc"]


def kernel(x, W, gamma, beta):
    """Full-input entry point: shard batch over 8 cores, run SPMD, gather."""
    import ml_dtypes
    from concourse.bass_utils import run_bass_kernel_spmd

    n_cores, nimg = 8, 4
    nc = _get_compiled()
    w2 = prep_w(np.asarray(W)).astype(ml_dtypes.bfloat16)
    g2 = np.ascontiguousarray(np.asarray(gamma, np.float32).reshape(K, 1))
    b2 = np.ascontiguousarray(np.asarray(beta, np.float32).reshape(K, 1))
    xb = np.asarray(x).astype(ml_dtypes.bfloat16)
    in_maps = [
        {
            "x": np.ascontiguousarray(xb[c * nimg : (c + 1) * nimg]),
            "w": w2,
            "gamma": g2,
            "beta": b2,
        }
        for c in range(n_cores)
    ]
    res = run_bass_kernel_spmd(nc, in_maps, core_ids=list(range(n_cores)))
    out = np.concatenate(
        [res.results[c]["out"] for c in range(n_cores)], axis=0
    ).astype(np.float32)
    return out


def run_traced(x, W, gamma, beta):
    """Like kernel() but with NTFF tracing; returns (out, BassKernelResults)."""
    import ml_dtypes
    from concourse.bass_utils import run_bass_kernel_spmd

    n_cores, nimg = 8, 4
    nc = _get_compiled()
    w2 = prep_w(np.asarray(W)).astype(ml_dtypes.bfloat16)
    g2 = np.ascontiguousarray(np.asarray(gamma, np.float32).reshape(K, 1))
    b2 = np.ascontiguousarray(np.asarray(beta, np.float32).reshape(K, 1))
    xb = np.asarray(x).astype(ml_dtypes.bfloat16)
    in_maps = [
        {
            "x": np.ascontiguousarray(xb[c * nimg : (c + 1) * nimg]),
            "w": w2,
            "gamma": g2,
            "beta": b2,
        }
        for c in range(n_cores)
    ]
    res = run_bass_kernel_spmd(nc, in_maps, core_ids=list(range(n_cores)), trace=True)
    out = np.concatenate(
        [res.results[c]["out"] for c in range(n_cores)], axis=0
    ).astype(np.float32)
    return out, res
